# revision 17
# baseline (speedup 1.0000x reference)
"""Trainium2 Bass kernel for nn_EquivariantMessagePasser (gnn_message_passing).

Strategy (8 NeuronCores, SPMD):
  - Atoms block-sharded: 1280/core (10240 padded). Pairs assigned to the core
    owning their center, grouped into 10 windows of 128 centers per core,
    each window padded to a uniform number B of 128-pair blocks.
  - _linear(x,U,W) == P @ x @ W with P = U@U.T, so both equivariant linears
    collapse into one matmul with kron(P, W) per l (g, MSG_SCALE folded in).
  - Per core: rmsnorm+linear for its own 1280 atoms -> x (bf16), AllGather to
    a full [10240, 960] atom table in DRAM (hidden behind MLP runahead);
    radial MLP (bf16 A, fp32r B, bf16 C; pairs-major output); indirect-DMA
    gather of x[neighbors]; messages = (sh@U.T outer r) * x_gathered (bf16);
    scatter-add via host-built one-hot bf16 matmul accumulated in PSUM per
    window; out = f + pooled @ kron(P, 0.1*Wout).
"""
import os
import sys
import numpy as np

for _p in ("/opt/trn_rl_repo", "/root/.axon_site/_ro/trn_rl_repo"):
    if os.path.isdir(_p) and _p not in sys.path:
        sys.path.insert(0, _p)

import ml_dtypes  # noqa: E402
import concourse.bass as bass  # noqa: E402
import concourse.tile as tile  # noqa: E402
from concourse import bacc, mybir  # noqa: E402
from concourse.bass_utils import run_bass_kernel_spmd  # noqa: E402
from concourse.masks import make_identity  # noqa: E402

F32 = mybir.dt.float32
F32R = mybir.dt.float32r
BF16 = mybir.dt.bfloat16
I32 = mybir.dt.int32

KL = [128, 96, 64, 32]
NL = [8, 8, 6, 4]
ML = [1, 3, 5, 7]
HL = [4 * k for k in KL]
NATOMS = 10000
NPAIRS = 100000
EPS = 1e-6
MSG_SCALE = 0.1
NCORES = 8
APC = 1280
NA_PAD = NCORES * APC
WPC = 10
WIN = 128
MKL = [m * k for m, k in zip(ML, KL)]          # 128, 288, 320, 224
MKOFF = [0, 128, 416, 736]
FTOT = 960
MKPAD = [256, 288, 320, 256]
CHUNKS = [[128], [128, 128, 32], [128, 128, 64], [128, 96]]
RBOFF = [0, 32, 64, 96]                        # rb/A row offsets (tile_position)
ROFF = [0, 128, 224, 288]
USHOFF = [0, 1, 4, 9]
RTOT = 320
USHTOT = 16


def _subslab_split(B):
    out = []
    rem = B
    while rem > 5:
        out.append(4)
        rem -= 4
    if rem == 5:
        out += [3, 2]
    else:
        out.append(rem)
    return out


def _host_prep(inputs):
    f = [np.asarray(inputs[f"f{l}"], np.float32) for l in range(4)]
    U = [np.asarray(inputs[f"U{l}"], np.float32) for l in range(4)]
    g = [np.asarray(inputs[f"g{l}"], np.float32) for l in range(4)]
    Wi = [np.asarray(inputs[f"Win{l}"], np.float32) for l in range(4)]
    Wo = [np.asarray(inputs[f"Wout{l}"], np.float32) for l in range(4)]
    A = [np.asarray(inputs[f"A{l}"], np.float32) for l in range(4)]
    Bm = [np.asarray(inputs[f"B{l}"], np.float32) for l in range(4)]
    C = [np.asarray(inputs[f"C{l}"], np.float32) for l in range(4)]
    rb = [np.asarray(inputs[f"rb{l}"], np.float32) for l in range(4)]
    sh = [np.asarray(inputs[f"sh{l}"], np.float32) for l in range(4)]
    centers = np.asarray(inputs["centers"], np.int64)
    neighbors = np.asarray(inputs["neighbors"], np.int64)

    P = [U[l] @ U[l].T for l in range(4)]
    bigwin = [np.kron(P[l], np.diag(g[l]) @ Wi[l]) for l in range(4)]
    bigwout = [MSG_SCALE * np.kron(P[l], Wo[l]) for l in range(4)]

    def pack_bigw(mats):
        cols = sum(len(CHUNKS[l]) * MKPAD[l] for l in range(4))
        out = np.zeros((128, cols), np.float32)
        offs = []
        c0 = 0
        for l in range(4):
            loffs = []
            r0 = 0
            for s in CHUNKS[l]:
                out[:s, c0:c0 + MKL[l]] = mats[l][r0:r0 + s, :]
                loffs.append(c0)
                r0 += s
                c0 += MKPAD[l]
            offs.append(loffs)
        return out, offs

    bigwin_pk, bw_offs = pack_bigw(bigwin)
    bigwout_pk, _ = pack_bigw(bigwout)
    bigwout_pk = bigwout_pk.astype(ml_dtypes.bfloat16)

    # A: [128, 1280] rows RBOFF[l]..+n_l (bf16)
    a_cols = sum(HL)
    a_pk = np.zeros((128, a_cols), np.float32)
    a_offs = []
    c0 = 0
    for l in range(4):
        a_pk[RBOFF[l]:RBOFF[l] + NL[l], c0:c0 + HL[l]] = A[l]
        a_offs.append(c0)
        c0 += HL[l]

    b_cols = sum((HL[l] // 128) ** 2 * 128 for l in range(4))
    b_pk = np.zeros((128, b_cols), np.float32)
    b_offs = {}
    c0 = 0
    for l in range(4):
        nch = HL[l] // 128
        for i in range(nch):
            for j in range(nch):
                b_pk[:, c0:c0 + 128] = Bm[l][i * 128:(i + 1) * 128,
                                             j * 128:(j + 1) * 128]
                b_offs[(l, i, j)] = c0
                c0 += 128
    c_cols = sum((HL[l] // 128) * KL[l] for l in range(4))
    c_pk = np.zeros((128, c_cols), np.float32)
    c_offs = {}
    c0 = 0
    for l in range(4):
        nch = HL[l] // 128
        for i in range(nch):
            c_pk[:, c0:c0 + KL[l]] = C[l][i * 128:(i + 1) * 128, :]
            c_offs[(l, i)] = c0
            c0 += KL[l]

    ush_all = np.concatenate([sh[l] @ U[l].T for l in range(4)], axis=1)
    rb_all = np.concatenate(rb, axis=1)  # [NPAIRS, 26]

    f_cat = np.zeros((NA_PAD, FTOT), np.float32)
    for l in range(4):
        f_cat[:NATOMS, MKOFF[l]:MKOFF[l] + MKL[l]] = f[l].reshape(NATOMS, MKL[l])

    core_of = centers // APC
    win_of = (centers % APC) // WIN
    pair_lists = [[[] for _ in range(WPC)] for _ in range(NCORES)]
    for p in range(NPAIRS):
        pair_lists[core_of[p]][win_of[p]].append(p)
    B = max(2, max((len(pl) + WIN - 1) // WIN
                   for cl in pair_lists for pl in cl))
    ss = _subslab_split(B)
    padp = WPC * B * WIN
    nblk = WPC * B

    nbr = np.zeros((NCORES, padp), np.int32)
    cof = np.zeros((NCORES, padp), np.int32)
    valid = np.zeros((NCORES, padp), bool)
    ush_pm = np.zeros((NCORES, padp, USHTOT), np.float32)
    rbT = np.zeros((NCORES, 128, padp), np.float32)
    for c in range(NCORES):
        for w in range(WPC):
            pl = np.asarray(pair_lists[c][w], np.int64)
            s0 = w * B * WIN
            n = len(pl)
            nbr[c, s0:s0 + n] = neighbors[pl]
            cof[c, s0:s0 + n] = centers[pl] % WIN
            valid[c, s0:s0 + n] = True
            ush_pm[c, s0:s0 + n] = ush_all[pl]
            rbt = rb_all[pl].T  # [26, n]
            for l in range(4):
                lo = [0, 8, 16, 22][l]
                rbT[c, RBOFF[l]:RBOFF[l] + NL[l], s0:s0 + n] = rbt[lo:lo + NL[l]]

    # one-hot sel per pair slot [padp, 128] bf16; invalid slots -> zero row
    sel_np = np.zeros((NCORES, padp, WIN), ml_dtypes.bfloat16)
    for c in range(NCORES):
        idxs = np.nonzero(valid[c])[0]
        sel_np[c][idxs, cof[c][idxs]] = 1.0

    invmk = np.tile(np.array([1.0 / mk for mk in MKL], np.float32), (128, 1))

    const_map = dict(
        w_bigwin=bigwin_pk, w_bigwout=bigwout_pk,
        w_a=a_pk.astype(ml_dtypes.bfloat16), w_b=b_pk,
        w_c=c_pk.astype(ml_dtypes.bfloat16), w_invmk=invmk)
    in_maps = []
    for c in range(NCORES):
        m = dict(const_map)
        m["in_f"] = f_cat[c * APC:(c + 1) * APC]
        m["in_rbt"] = rbT[c].astype(ml_dtypes.bfloat16)
        m["in_ush"] = ush_pm[c].reshape(padp * USHTOT).astype(ml_dtypes.bfloat16)
        m["in_nbr"] = nbr[c]
        m["in_sel"] = sel_np[c].reshape(padp * WIN)
        in_maps.append(m)

    meta = dict(B=B, ss=ss, padp=padp, nblk=nblk, bw_offs=bw_offs,
                a_offs=a_offs, b_offs=b_offs, c_offs=c_offs,
                a_cols=a_cols, b_cols=b_cols, c_cols=c_cols,
                bigw_cols=bigwin_pk.shape[1])
    return in_maps, meta


def _build_program(meta):
    B = meta["B"]
    ss = meta["ss"]
    padp = meta["padp"]
    nblk = meta["nblk"]
    bw_offs = meta["bw_offs"]
    a_offs = meta["a_offs"]
    b_offs = meta["b_offs"]
    c_offs = meta["c_offs"]

    nc = bacc.Bacc("TRN2", target_bir_lowering=False, debug=False,
                   num_devices=NCORES)

    in_f = nc.dram_tensor("in_f", [APC, FTOT], F32, kind="ExternalInput")
    in_rbt = nc.dram_tensor("in_rbt", [128, padp], BF16, kind="ExternalInput")
    in_ush = nc.dram_tensor("in_ush", [padp * USHTOT], BF16, kind="ExternalInput")
    in_nbr = nc.dram_tensor("in_nbr", [padp], I32, kind="ExternalInput")
    in_sel = nc.dram_tensor("in_sel", [padp * WIN], BF16, kind="ExternalInput")
    w_bigwin = nc.dram_tensor("w_bigwin", [128, meta["bigw_cols"]], F32R,
                              kind="ExternalInput")
    w_bigwout = nc.dram_tensor("w_bigwout", [128, meta["bigw_cols"]], BF16,
                               kind="ExternalInput")
    w_a = nc.dram_tensor("w_a", [128, meta["a_cols"]], BF16, kind="ExternalInput")
    w_b = nc.dram_tensor("w_b", [128, meta["b_cols"]], F32R, kind="ExternalInput")
    w_c = nc.dram_tensor("w_c", [128, meta["c_cols"]], BF16, kind="ExternalInput")
    w_invmk = nc.dram_tensor("w_invmk", [128, 4], F32, kind="ExternalInput")

    out_d = nc.dram_tensor("out", [APC, FTOT], F32, kind="ExternalOutput")

    x_local = nc.dram_tensor("x_local", [APC, FTOT], BF16)
    x_full = nc.dram_tensor("x_full", [NA_PAD, FTOT], BF16, addr_space="Shared")
    pooled_dram = nc.dram_tensor("pooled_dram", [APC, FTOT], BF16)

    SILU = mybir.ActivationFunctionType.Silu
    SQRT = mybir.ActivationFunctionType.Sqrt
    SQUARE = mybir.ActivationFunctionType.Square
    MUL = mybir.AluOpType.mult

    from contextlib import ExitStack
    with tile.TileContext(nc) as tc:
        with ExitStack() as root:
            const = root.enter_context(tc.tile_pool(name="const", bufs=1))
            bigwin_t = const.tile([128, meta["bigw_cols"]], F32R)
            nc.sync.dma_start(bigwin_t[:], w_bigwin[:])
            bigwout_t = const.tile([128, meta["bigw_cols"]], BF16)
            nc.sync.dma_start(bigwout_t[:], w_bigwout[:])
            invmk_t = const.tile([128, 4], F32)
            nc.sync.dma_start(invmk_t[:], w_invmk[:])
            eps_t = const.tile([128, 1], F32)
            nc.gpsimd.memset(eps_t[:], EPS)
            ident_t = const.tile([128, 128], F32)
            make_identity(nc, ident_t[:])
            ident_bf = const.tile([128, 128], BF16)
            nc.vector.tensor_copy(ident_bf[:], ident_t[:])

            # phase-2 SBUF pools created before the collective critical
            ph2 = ExitStack()
            h1p = ph2.enter_context(tc.tile_pool(name="h1p", bufs=2))
            h2p = ph2.enter_context(tc.tile_pool(name="h2p", bufs=5))
            rp = ph2.enter_context(tc.tile_pool(name="rp", bufs=46))
            xgp = ph2.enter_context(tc.tile_pool(name="xgp", bufs=3))
            uvp = ph2.enter_context(tc.tile_pool(name="uvp", bufs=3))
            msp = ph2.enter_context(tc.tile_pool(name="msp", bufs=3))
            selp = ph2.enter_context(tc.tile_pool(name="selp", bufs=3))
            pbp = ph2.enter_context(tc.tile_pool(name="pbp", bufs=2))

            # ---------------- phase 1: x = rmsnorm + linear for local atoms
            with ExitStack() as ph1:
                xp = ph1.enter_context(tc.tile_pool(name="xp", bufs=2))
                xps = ph1.enter_context(
                    tc.tile_pool(name="xps", bufs=4, space="PSUM"))
                for a in range(APC // 128):
                    f_t = xp.tile([128, FTOT], F32, tag="f")
                    nc.sync.dma_start(f_t[:], in_f[a * 128:(a + 1) * 128, :])
                    scr = xp.tile([128, RTOT], F32, tag="scr")
                    ms = xp.tile([128, 4], F32, tag="ms")
                    for l in range(4):
                        mk = MKL[l]
                        nc.scalar.activation(
                            scr[:, :mk], f_t[:, MKOFF[l]:MKOFF[l] + mk],
                            SQUARE, accum_out=ms[:, l:l + 1])
                    ms2 = xp.tile([128, 4], F32, tag="ms2")
                    nc.vector.tensor_tensor(out=ms2[:], in0=ms[:],
                                            in1=invmk_t[:], op=MUL)
                    sd = xp.tile([128, 4], F32, tag="sd")
                    nc.scalar.activation(sd[:], ms2[:], SQRT,
                                         bias=eps_t[:, :1])
                    rs = xp.tile([128, 4], F32, tag="rs")
                    nc.vector.reciprocal(rs[:], sd[:])
                    for l in range(4):
                        mk = MKL[l]
                        nc.vector.tensor_tensor(
                            out=f_t[:, MKOFF[l]:MKOFF[l] + mk],
                            in0=f_t[:, MKOFF[l]:MKOFF[l] + mk],
                            in1=rs[:, l:l + 1].to_broadcast([128, mk]),
                            op=MUL)
                    fhT = xp.tile([128, 9 * 128], F32R, tag="fhT")
                    ci = 0
                    for l in range(4):
                        r0 = MKOFF[l]
                        for s in CHUNKS[l]:
                            tp = xps.tile([128, 128], F32, tag="tp")
                            nc.tensor.transpose(out=tp[:s, :],
                                                in_=f_t[:, r0:r0 + s],
                                                identity=ident_t[:])
                            nc.vector.tensor_copy(
                                fhT[:s, ci * 128:(ci + 1) * 128], tp[:s, :])
                            r0 += s
                            ci += 1
                    x_sb = xp.tile([128, FTOT], BF16, tag="x_sb")
                    ci = 0
                    for l in range(4):
                        xo = xps.tile([128, 512], F32, tag="xo")
                        for k, s in enumerate(CHUNKS[l]):
                            nc.tensor.matmul(
                                xo[:, :MKPAD[l]],
                                lhsT=fhT[:s, ci * 128:ci * 128 + 128],
                                rhs=bigwin_t[:s, bw_offs[l][k]:
                                             bw_offs[l][k] + MKPAD[l]],
                                start=(k == 0), stop=(k == len(CHUNKS[l]) - 1))
                            ci += 1
                        nc.scalar.copy(
                            x_sb[:, MKOFF[l]:MKOFF[l] + MKL[l]],
                            xo[:, :MKL[l]])
                    nc.sync.dma_start(x_local[a * 128:(a + 1) * 128, :],
                                      x_sb[:])

            # MLP-only constants loaded after the x-phase DMAs (keeps the
            # sync queue free for f/x traffic during phase 1)
            a_t = const.tile([128, meta["a_cols"]], BF16)
            nc.sync.dma_start(a_t[:], w_a[:])
            b_t = const.tile([128, meta["b_cols"]], F32R)
            nc.sync.dma_start(b_t[:], w_b[:])
            c_t = const.tile([128, meta["c_cols"]], BF16)
            nc.sync.dma_start(c_t[:], w_c[:])
            rbt_t = const.tile([128, padp], BF16)
            nc.sync.dma_start(rbt_t[:], in_rbt[:])
            ush_t = const.tile([128, nblk, USHTOT], BF16)
            nc.sync.dma_start(
                ush_t[:],
                in_ush[:].rearrange("(x p c) -> p x c", p=128, c=USHTOT))
            idx_t = const.tile([128, nblk], I32)
            nc.sync.dma_start(
                idx_t[:], in_nbr[:].rearrange("(x p) -> p x", p=128))

            mlpps = ph2.enter_context(
                tc.tile_pool(name="mlpps", bufs=2, space="PSUM"))
            cps = ph2.enter_context(
                tc.tile_pool(name="cps", bufs=2, space="PSUM"))
            plps = ph2.enter_context(
                tc.tile_pool(name="plps", bufs=1, space="PSUM"))

            # ---------------- phase 2: MLP + messages + scatter
            # software-pipelined: window w's messages are emitted after
            # window (w+RA)'s MLP so the per-engine queues hold ~RA windows
            # of AG-independent work while the AllGather runs.
            RA = 3
            r_tiles = {}

            def emit_mlp(w):
                s0b = w * B
                blk0 = 0
                rlist = []
                for sbc in ss:
                    npair = sbc * 128
                    pr0 = (s0b + blk0) * 128
                    h2_l = []
                    for l in range(4):
                        nch = HL[l] // 128
                        h1 = h1p.tile([128, nch, 512], F32R, tag="h1")
                        for g0 in range(0, nch, 2):
                            gsz = min(2, nch - g0)
                            aps = mlpps.tile([128, 2, 512], F32, tag="mlp")
                            for jj in range(gsz):
                                j = g0 + jj
                                nc.tensor.matmul(
                                    aps[:, jj, :npair],
                                    lhsT=a_t[RBOFF[l]:RBOFF[l] + NL[l],
                                             a_offs[l] + j * 128:
                                             a_offs[l] + (j + 1) * 128],
                                    rhs=rbt_t[RBOFF[l]:RBOFF[l] + NL[l],
                                              pr0:pr0 + npair],
                                    start=True, stop=True,
                                    tile_position=(RBOFF[l], 0))
                            nc.scalar.activation(
                                h1[:, g0:g0 + gsz, :npair],
                                aps[:, :gsz, :npair], SILU)
                        h2 = h2p.tile([128, nch, 512], BF16, tag="h2")
                        for g0 in range(0, nch, 2):
                            gsz = min(2, nch - g0)
                            bps = mlpps.tile([128, 2, 512], F32, tag="mlp")
                            for jj in range(gsz):
                                j = g0 + jj
                                for i in range(nch):
                                    nc.tensor.matmul(
                                        bps[:, jj, :npair],
                                        lhsT=b_t[:, b_offs[(l, i, j)]:
                                                 b_offs[(l, i, j)] + 128],
                                        rhs=h1[:, i, :npair],
                                        start=(i == 0), stop=(i == nch - 1))
                            nc.scalar.activation(
                                h2[:, g0:g0 + gsz, :npair],
                                bps[:, :gsz, :npair], SILU)
                        h2_l.append(h2)
                    for b in range(sbc):
                        rps = cps.tile([128, 512], F32, tag="cps")
                        for l in range(4):
                            nch = HL[l] // 128
                            for i in range(nch):
                                nc.tensor.matmul(
                                    rps[:, ROFF[l]:ROFF[l] + KL[l]],
                                    lhsT=h2_l[l][:, i, b * 128:(b + 1) * 128],
                                    rhs=c_t[:, c_offs[(l, i)]:
                                            c_offs[(l, i)] + KL[l]],
                                    start=(i == 0), stop=(i == nch - 1))
                        r_t = rp.tile([128, RTOT], BF16, tag="r")
                        nc.vector.tensor_copy(r_t[:], rps[:, :RTOT])
                        rlist.append(r_t)
                    blk0 += sbc
                r_tiles[w] = rlist

            def emit_msg(w):
                s0b = w * B
                sel_w = selp.tile([128, B, WIN], BF16, tag="sel")
                nc.sync.dma_start(
                    sel_w[:],
                    in_sel[s0b * WIN * 128:(s0b + B) * WIN * 128].rearrange(
                        "(b p c) -> p b c", p=128, c=WIN))
                pooled = plps.tile([128, 1024], F32, tag="pooled")
                for bb in range(B):
                    wb = s0b + bb
                    r_t = r_tiles[w][bb]
                    xg = xgp.tile([128, FTOT], BF16, tag="xg")
                    nc.gpsimd.indirect_dma_start(
                        out=xg[:], out_offset=None, in_=x_full[:],
                        in_offset=bass.IndirectOffsetOnAxis(
                            ap=idx_t[:, wb:wb + 1], axis=0))
                    uve = uvp.tile([128, FTOT], BF16, tag="uve")
                    for l in range(4):
                        m, k = ML[l], KL[l]
                        nc.vector.tensor_tensor(
                            out=uve[:, MKOFF[l]:MKOFF[l] + m * k]
                                .rearrange("p (u k) -> p u k", u=m),
                            in0=ush_t[:, wb, USHOFF[l]:USHOFF[l] + m]
                                .unsqueeze(2).to_broadcast([128, m, k]),
                            in1=r_t[:, ROFF[l]:ROFF[l] + k]
                                .unsqueeze(1).to_broadcast([128, m, k]),
                            op=MUL)
                    msg = msp.tile([128, FTOT], BF16, tag="msg")
                    nc.vector.tensor_tensor(out=msg[:], in0=uve[:],
                                            in1=xg[:], op=MUL)
                    first = (bb == 0)
                    last = (bb == B - 1)
                    nc.tensor.matmul(pooled[:, 0:512],
                                     lhsT=sel_w[:, bb, :],
                                     rhs=msg[:, 0:512],
                                     start=first, stop=last)
                    nc.tensor.matmul(pooled[:, 512:512 + 448],
                                     lhsT=sel_w[:, bb, :],
                                     rhs=msg[:, 512:960],
                                     start=first, stop=last)
                del r_tiles[w]
                pb = pbp.tile([128, FTOT], BF16, tag="pb")
                nc.vector.tensor_copy(pb[:], pooled[:, :FTOT])
                nc.sync.dma_start(pooled_dram[w * 128:(w + 1) * 128, :], pb[:])

            # 3 windows of MLP queued ahead of the collective barrier so
            # PE/ACT/DVE stay busy while the AllGather runs
            for w in range(RA):
                emit_mlp(w)

            # ---------------- allgather x (hidden behind MLP runahead)
            with tc.tile_critical():
                with nc.semaphore("cc_sem") as cc_sem:
                    nc.gpsimd.collective_compute(
                        "AllGather", mybir.AluOpType.bypass,
                        ins=[x_local[:]], outs=[x_full[:]],
                        replica_groups=[list(range(NCORES))],
                    ).then_inc(cc_sem)
                    nc.gpsimd.wait_ge(cc_sem, 1)

            for w in range(RA, WPC):
                emit_mlp(w)
                emit_msg(w - RA)
            for w in range(WPC - RA, WPC):
                emit_msg(w)
            ph2.close()

            # ---------------- phase 3: out = f + pooled @ BigWout
            with ExitStack() as ph3:
                op = ph3.enter_context(tc.tile_pool(name="op", bufs=3))
                ops = ph3.enter_context(
                    tc.tile_pool(name="ops", bufs=4, space="PSUM"))
                for w in range(WPC):
                    f_t = op.tile([128, FTOT], F32, tag="fo")
                    nc.sync.dma_start(f_t[:], in_f[w * 128:(w + 1) * 128, :])
                    pl_t = op.tile([128, FTOT], BF16, tag="pl")
                    nc.sync.dma_start(pl_t[:],
                                      pooled_dram[w * 128:(w + 1) * 128, :])
                    pT = op.tile([128, 9 * 128], BF16, tag="pT")
                    ci = 0
                    for l in range(4):
                        r0 = MKOFF[l]
                        for s in CHUNKS[l]:
                            tp = ops.tile([128, 128], BF16, tag="tp2")
                            nc.tensor.transpose(out=tp[:s, :],
                                                in_=pl_t[:, r0:r0 + s],
                                                identity=ident_bf[:])
                            nc.vector.tensor_copy(
                                pT[:s, ci * 128:(ci + 1) * 128], tp[:s, :])
                            r0 += s
                            ci += 1
                    out_t = op.tile([128, FTOT], F32, tag="out")
                    ci = 0
                    for l in range(4):
                        yo = ops.tile([128, 512], F32, tag="yo")
                        for k, s in enumerate(CHUNKS[l]):
                            nc.tensor.matmul(
                                yo[:, :MKPAD[l]],
                                lhsT=pT[:s, ci * 128:ci * 128 + 128],
                                rhs=bigwout_t[:s, bw_offs[l][k]:
                                              bw_offs[l][k] + MKPAD[l]],
                                start=(k == 0), stop=(k == len(CHUNKS[l]) - 1))
                            ci += 1
                        nc.vector.tensor_add(
                            out_t[:, MKOFF[l]:MKOFF[l] + MKL[l]],
                            f_t[:, MKOFF[l]:MKOFF[l] + MKL[l]],
                            yo[:, :MKL[l]])
                    nc.sync.dma_start(out_d[w * 128:(w + 1) * 128, :],
                                      out_t[:])

    nc.compile()
    return nc


def kernel(**inputs):
    in_maps, meta = _host_prep(inputs)
    nc = _build_program(meta)
    trace = bool(int(os.environ.get("KERNEL_TRACE", "0")))
    res = run_bass_kernel_spmd(nc, in_maps, list(range(NCORES)), trace=trace)
    if trace and res.exec_time_ns is not None:
        print(f"HW exec time: {res.exec_time_ns} ns")
        kernel.last_exec_time_ns = res.exec_time_ns
    full = np.concatenate([res.results[c]["out"] for c in range(NCORES)],
                          axis=0)[:NATOMS]
    outs = []
    for l in range(4):
        outs.append(np.ascontiguousarray(
            full[:, MKOFF[l]:MKOFF[l] + MKL[l]].reshape(
                NATOMS, ML[l], KL[l]).astype(np.float32)))
    return tuple(outs)


# revision 19
# speedup vs baseline: 1.0684x; 1.0684x over previous
"""Trainium2 Bass kernel for nn_EquivariantMessagePasser (gnn_message_passing).

Strategy (8 NeuronCores, SPMD):
  - Atoms block-sharded: 1280/core (10240 padded). Pairs assigned to the core
    owning their center, grouped into 10 windows of 128 centers per core,
    each window padded to a uniform number B of 128-pair blocks.
  - _linear(x,U,W) == P @ x @ W with P = U@U.T, so both equivariant linears
    collapse into one matmul with kron(P, W) per l (g, MSG_SCALE folded in).
  - Per core: rmsnorm+linear for its own 1280 atoms -> x (bf16), AllGather to
    a full [10240, 960] atom table in DRAM (hidden behind MLP runahead);
    radial MLP (bf16 A, fp32r B, bf16 C; pairs-major output); indirect-DMA
    gather of x[neighbors]; messages = (sh@U.T outer r) * x_gathered (bf16);
    scatter-add via host-built one-hot bf16 matmul accumulated in PSUM per
    window; out = f + pooled @ kron(P, 0.1*Wout).
"""
import os
import sys
import numpy as np

for _p in ("/opt/trn_rl_repo", "/root/.axon_site/_ro/trn_rl_repo"):
    if os.path.isdir(_p) and _p not in sys.path:
        sys.path.insert(0, _p)

import ml_dtypes  # noqa: E402
import concourse.bass as bass  # noqa: E402
import concourse.tile as tile  # noqa: E402
from concourse import bacc, mybir  # noqa: E402
from concourse.bass_utils import run_bass_kernel_spmd  # noqa: E402
from concourse.masks import make_identity  # noqa: E402

F32 = mybir.dt.float32
F32R = mybir.dt.float32r
BF16 = mybir.dt.bfloat16
I32 = mybir.dt.int32

KL = [128, 96, 64, 32]
NL = [8, 8, 6, 4]
ML = [1, 3, 5, 7]
HL = [4 * k for k in KL]
NATOMS = 10000
NPAIRS = 100000
EPS = 1e-6
MSG_SCALE = 0.1
NCORES = 8
APC = 1280
NA_PAD = NCORES * APC
WPC = 10
WIN = 128
MKL = [m * k for m, k in zip(ML, KL)]          # 128, 288, 320, 224
MKOFF = [0, 128, 416, 736]
FTOT = 960
MKPAD = [256, 288, 320, 256]
CHUNKS = [[128], [128, 128, 32], [128, 128, 64], [128, 96]]
RBOFF = [0, 32, 64, 96]                        # rb/A row offsets (tile_position)
ROFF = [0, 128, 224, 288]
USHOFF = [0, 1, 4, 9]
RTOT = 320
USHTOT = 16


def _subslab_split(B):
    out = []
    rem = B
    while rem > 5:
        out.append(4)
        rem -= 4
    if rem == 5:
        out += [3, 2]
    else:
        out.append(rem)
    return out


def _host_prep(inputs):
    f = [np.asarray(inputs[f"f{l}"], np.float32) for l in range(4)]
    U = [np.asarray(inputs[f"U{l}"], np.float32) for l in range(4)]
    g = [np.asarray(inputs[f"g{l}"], np.float32) for l in range(4)]
    Wi = [np.asarray(inputs[f"Win{l}"], np.float32) for l in range(4)]
    Wo = [np.asarray(inputs[f"Wout{l}"], np.float32) for l in range(4)]
    A = [np.asarray(inputs[f"A{l}"], np.float32) for l in range(4)]
    Bm = [np.asarray(inputs[f"B{l}"], np.float32) for l in range(4)]
    C = [np.asarray(inputs[f"C{l}"], np.float32) for l in range(4)]
    rb = [np.asarray(inputs[f"rb{l}"], np.float32) for l in range(4)]
    sh = [np.asarray(inputs[f"sh{l}"], np.float32) for l in range(4)]
    centers = np.asarray(inputs["centers"], np.int64)
    neighbors = np.asarray(inputs["neighbors"], np.int64)

    P = [U[l] @ U[l].T for l in range(4)]
    bigwin = [np.kron(P[l], np.diag(g[l]) @ Wi[l]) for l in range(4)]
    bigwout = [MSG_SCALE * np.kron(P[l], Wo[l]) for l in range(4)]

    def pack_bigw(mats):
        cols = sum(len(CHUNKS[l]) * MKPAD[l] for l in range(4))
        out = np.zeros((128, cols), np.float32)
        offs = []
        c0 = 0
        for l in range(4):
            loffs = []
            r0 = 0
            for s in CHUNKS[l]:
                out[:s, c0:c0 + MKL[l]] = mats[l][r0:r0 + s, :]
                loffs.append(c0)
                r0 += s
                c0 += MKPAD[l]
            offs.append(loffs)
        return out, offs

    bigwin_pk, bw_offs = pack_bigw(bigwin)
    bigwout_pk, _ = pack_bigw(bigwout)
    bigwout_pk = bigwout_pk.astype(ml_dtypes.bfloat16)

    # A: [128, 1280] rows RBOFF[l]..+n_l (bf16)
    a_cols = sum(HL)
    a_pk = np.zeros((128, a_cols), np.float32)
    a_offs = []
    c0 = 0
    for l in range(4):
        a_pk[RBOFF[l]:RBOFF[l] + NL[l], c0:c0 + HL[l]] = A[l]
        a_offs.append(c0)
        c0 += HL[l]

    b_cols = sum((HL[l] // 128) ** 2 * 128 for l in range(4))
    b_pk = np.zeros((128, b_cols), np.float32)
    b_offs = {}
    c0 = 0
    for l in range(4):
        nch = HL[l] // 128
        for i in range(nch):
            for j in range(nch):
                b_pk[:, c0:c0 + 128] = Bm[l][i * 128:(i + 1) * 128,
                                             j * 128:(j + 1) * 128]
                b_offs[(l, i, j)] = c0
                c0 += 128
    c_cols = sum((HL[l] // 128) * KL[l] for l in range(4))
    c_pk = np.zeros((128, c_cols), np.float32)
    c_offs = {}
    c0 = 0
    for l in range(4):
        nch = HL[l] // 128
        for i in range(nch):
            c_pk[:, c0:c0 + KL[l]] = C[l][i * 128:(i + 1) * 128, :]
            c_offs[(l, i)] = c0
            c0 += KL[l]

    ush_all = np.concatenate([sh[l] @ U[l].T for l in range(4)], axis=1)
    rb_all = np.concatenate(rb, axis=1)  # [NPAIRS, 26]

    f_cat = np.zeros((NA_PAD, FTOT), np.float32)
    for l in range(4):
        f_cat[:NATOMS, MKOFF[l]:MKOFF[l] + MKL[l]] = f[l].reshape(NATOMS, MKL[l])

    core_of = centers // APC
    win_of = (centers % APC) // WIN
    pair_lists = [[[] for _ in range(WPC)] for _ in range(NCORES)]
    for p in range(NPAIRS):
        pair_lists[core_of[p]][win_of[p]].append(p)
    B = max(2, max((len(pl) + WIN - 1) // WIN
                   for cl in pair_lists for pl in cl))
    ss = _subslab_split(B)
    padp = WPC * B * WIN
    nblk = WPC * B

    nbr = np.zeros((NCORES, padp), np.int32)
    cof = np.zeros((NCORES, padp), np.int32)
    valid = np.zeros((NCORES, padp), bool)
    ush_pm = np.zeros((NCORES, padp, USHTOT), np.float32)
    rbT = np.zeros((NCORES, 128, padp), np.float32)
    for c in range(NCORES):
        for w in range(WPC):
            pl = np.asarray(pair_lists[c][w], np.int64)
            s0 = w * B * WIN
            n = len(pl)
            nbr[c, s0:s0 + n] = neighbors[pl]
            cof[c, s0:s0 + n] = centers[pl] % WIN
            valid[c, s0:s0 + n] = True
            ush_pm[c, s0:s0 + n] = ush_all[pl]
            rbt = rb_all[pl].T  # [26, n]
            for l in range(4):
                lo = [0, 8, 16, 22][l]
                rbT[c, RBOFF[l]:RBOFF[l] + NL[l], s0:s0 + n] = rbt[lo:lo + NL[l]]

    # one-hot sel per pair slot [padp, 128] bf16; invalid slots -> zero row
    sel_np = np.zeros((NCORES, padp, WIN), ml_dtypes.bfloat16)
    for c in range(NCORES):
        idxs = np.nonzero(valid[c])[0]
        sel_np[c][idxs, cof[c][idxs]] = 1.0

    invmk = np.tile(np.array([1.0 / mk for mk in MKL], np.float32), (128, 1))

    const_map = dict(
        w_bigwin=bigwin_pk, w_bigwout=bigwout_pk,
        w_a=a_pk.astype(ml_dtypes.bfloat16),
        w_b=b_pk.astype(ml_dtypes.bfloat16),
        w_c=c_pk.astype(ml_dtypes.bfloat16), w_invmk=invmk)
    in_maps = []
    for c in range(NCORES):
        m = dict(const_map)
        m["in_f"] = f_cat[c * APC:(c + 1) * APC]
        m["in_rbt"] = rbT[c].astype(ml_dtypes.bfloat16)
        m["in_ush"] = ush_pm[c].reshape(padp * USHTOT).astype(ml_dtypes.bfloat16)
        m["in_nbr"] = nbr[c]
        m["in_sel"] = sel_np[c].reshape(padp * WIN)
        in_maps.append(m)

    meta = dict(B=B, ss=ss, padp=padp, nblk=nblk, bw_offs=bw_offs,
                a_offs=a_offs, b_offs=b_offs, c_offs=c_offs,
                a_cols=a_cols, b_cols=b_cols, c_cols=c_cols,
                bigw_cols=bigwin_pk.shape[1])
    return in_maps, meta


def _build_program(meta):
    B = meta["B"]
    ss = meta["ss"]
    padp = meta["padp"]
    nblk = meta["nblk"]
    bw_offs = meta["bw_offs"]
    a_offs = meta["a_offs"]
    b_offs = meta["b_offs"]
    c_offs = meta["c_offs"]

    nc = bacc.Bacc("TRN2", target_bir_lowering=False, debug=False,
                   num_devices=NCORES)

    in_f = nc.dram_tensor("in_f", [APC, FTOT], F32, kind="ExternalInput")
    in_rbt = nc.dram_tensor("in_rbt", [128, padp], BF16, kind="ExternalInput")
    in_ush = nc.dram_tensor("in_ush", [padp * USHTOT], BF16, kind="ExternalInput")
    in_nbr = nc.dram_tensor("in_nbr", [padp], I32, kind="ExternalInput")
    in_sel = nc.dram_tensor("in_sel", [padp * WIN], BF16, kind="ExternalInput")
    w_bigwin = nc.dram_tensor("w_bigwin", [128, meta["bigw_cols"]], F32R,
                              kind="ExternalInput")
    w_bigwout = nc.dram_tensor("w_bigwout", [128, meta["bigw_cols"]], BF16,
                               kind="ExternalInput")
    w_a = nc.dram_tensor("w_a", [128, meta["a_cols"]], BF16, kind="ExternalInput")
    w_b = nc.dram_tensor("w_b", [128, meta["b_cols"]], BF16, kind="ExternalInput")
    w_c = nc.dram_tensor("w_c", [128, meta["c_cols"]], BF16, kind="ExternalInput")
    w_invmk = nc.dram_tensor("w_invmk", [128, 4], F32, kind="ExternalInput")

    out_d = nc.dram_tensor("out", [APC, FTOT], F32, kind="ExternalOutput")

    x_local = nc.dram_tensor("x_local", [APC, FTOT], BF16)
    x_full = nc.dram_tensor("x_full", [NA_PAD, FTOT], BF16, addr_space="Shared")
    pooled_dram = nc.dram_tensor("pooled_dram", [APC, FTOT], BF16)

    SILU = mybir.ActivationFunctionType.Silu
    SQRT = mybir.ActivationFunctionType.Sqrt
    SQUARE = mybir.ActivationFunctionType.Square
    MUL = mybir.AluOpType.mult

    from contextlib import ExitStack
    with tile.TileContext(nc) as tc:
        with ExitStack() as root:
            const = root.enter_context(tc.tile_pool(name="const", bufs=1))
            bigwin_t = const.tile([128, meta["bigw_cols"]], F32R)
            nc.sync.dma_start(bigwin_t[:], w_bigwin[:])
            bigwout_t = const.tile([128, meta["bigw_cols"]], BF16)
            nc.sync.dma_start(bigwout_t[:], w_bigwout[:])
            invmk_t = const.tile([128, 4], F32)
            nc.sync.dma_start(invmk_t[:], w_invmk[:])
            eps_t = const.tile([128, 1], F32)
            nc.gpsimd.memset(eps_t[:], EPS)
            ident_t = const.tile([128, 128], F32)
            make_identity(nc, ident_t[:])
            ident_bf = const.tile([128, 128], BF16)
            nc.vector.tensor_copy(ident_bf[:], ident_t[:])

            # phase-2 SBUF pools created before the collective critical
            ph2 = ExitStack()
            h1p = ph2.enter_context(tc.tile_pool(name="h1p", bufs=2))
            h2p = ph2.enter_context(tc.tile_pool(name="h2p", bufs=6))
            rp = ph2.enter_context(tc.tile_pool(name="rp", bufs=34))
            xgp = ph2.enter_context(tc.tile_pool(name="xgp", bufs=4))
            uvp = ph2.enter_context(tc.tile_pool(name="uvp", bufs=3))
            msp = ph2.enter_context(tc.tile_pool(name="msp", bufs=3))
            selp = ph2.enter_context(tc.tile_pool(name="selp", bufs=3))
            pbp = ph2.enter_context(tc.tile_pool(name="pbp", bufs=2))

            # ---------------- phase 1: x = rmsnorm + linear for local atoms
            with ExitStack() as ph1:
                xp = ph1.enter_context(tc.tile_pool(name="xp", bufs=2))
                xps = ph1.enter_context(
                    tc.tile_pool(name="xps", bufs=4, space="PSUM"))
                for a in range(APC // 128):
                    f_t = xp.tile([128, FTOT], F32, tag="f")
                    nc.sync.dma_start(f_t[:], in_f[a * 128:(a + 1) * 128, :])
                    scr = xp.tile([128, RTOT], F32, tag="scr")
                    ms = xp.tile([128, 4], F32, tag="ms")
                    for l in range(4):
                        mk = MKL[l]
                        nc.scalar.activation(
                            scr[:, :mk], f_t[:, MKOFF[l]:MKOFF[l] + mk],
                            SQUARE, accum_out=ms[:, l:l + 1])
                    ms2 = xp.tile([128, 4], F32, tag="ms2")
                    nc.vector.tensor_tensor(out=ms2[:], in0=ms[:],
                                            in1=invmk_t[:], op=MUL)
                    sd = xp.tile([128, 4], F32, tag="sd")
                    nc.scalar.activation(sd[:], ms2[:], SQRT,
                                         bias=eps_t[:, :1])
                    rs = xp.tile([128, 4], F32, tag="rs")
                    nc.vector.reciprocal(rs[:], sd[:])
                    for l in range(4):
                        mk = MKL[l]
                        nc.vector.tensor_tensor(
                            out=f_t[:, MKOFF[l]:MKOFF[l] + mk],
                            in0=f_t[:, MKOFF[l]:MKOFF[l] + mk],
                            in1=rs[:, l:l + 1].to_broadcast([128, mk]),
                            op=MUL)
                    fhT = xp.tile([128, 9 * 128], F32R, tag="fhT")
                    ci = 0
                    for l in range(4):
                        r0 = MKOFF[l]
                        for s in CHUNKS[l]:
                            tp = xps.tile([128, 128], F32, tag="tp")
                            nc.tensor.transpose(out=tp[:s, :],
                                                in_=f_t[:, r0:r0 + s],
                                                identity=ident_t[:])
                            nc.vector.tensor_copy(
                                fhT[:s, ci * 128:(ci + 1) * 128], tp[:s, :])
                            r0 += s
                            ci += 1
                    x_sb = xp.tile([128, FTOT], BF16, tag="x_sb")
                    ci = 0
                    for l in range(4):
                        xo = xps.tile([128, 512], F32, tag="xo")
                        for k, s in enumerate(CHUNKS[l]):
                            nc.tensor.matmul(
                                xo[:, :MKPAD[l]],
                                lhsT=fhT[:s, ci * 128:ci * 128 + 128],
                                rhs=bigwin_t[:s, bw_offs[l][k]:
                                             bw_offs[l][k] + MKPAD[l]],
                                start=(k == 0), stop=(k == len(CHUNKS[l]) - 1))
                            ci += 1
                        nc.scalar.copy(
                            x_sb[:, MKOFF[l]:MKOFF[l] + MKL[l]],
                            xo[:, :MKL[l]])
                    nc.sync.dma_start(x_local[a * 128:(a + 1) * 128, :],
                                      x_sb[:])

            # MLP-only constants loaded after the x-phase DMAs (keeps the
            # sync queue free for f/x traffic during phase 1)
            a_t = const.tile([128, meta["a_cols"]], BF16)
            nc.sync.dma_start(a_t[:], w_a[:])
            b_t = const.tile([128, meta["b_cols"]], BF16)
            nc.sync.dma_start(b_t[:], w_b[:])
            c_t = const.tile([128, meta["c_cols"]], BF16)
            nc.sync.dma_start(c_t[:], w_c[:])
            rbt_t = const.tile([128, padp], BF16)
            nc.sync.dma_start(rbt_t[:], in_rbt[:])
            ush_t = const.tile([128, nblk, USHTOT], BF16)
            nc.sync.dma_start(
                ush_t[:],
                in_ush[:].rearrange("(x p c) -> p x c", p=128, c=USHTOT))
            idx_t = const.tile([128, nblk], I32)
            nc.sync.dma_start(
                idx_t[:], in_nbr[:].rearrange("(x p) -> p x", p=128))

            mlpps = ph2.enter_context(
                tc.tile_pool(name="mlpps", bufs=2, space="PSUM"))
            cps = ph2.enter_context(
                tc.tile_pool(name="cps", bufs=2, space="PSUM"))
            plps = ph2.enter_context(
                tc.tile_pool(name="plps", bufs=1, space="PSUM"))

            # ---------------- allgather x (hidden behind MLP runahead)
            with tc.tile_critical():
                with nc.semaphore("cc_sem") as cc_sem:
                    nc.gpsimd.collective_compute(
                        "AllGather", mybir.AluOpType.bypass,
                        ins=[x_local[:]], outs=[x_full[:]],
                        replica_groups=[list(range(NCORES))],
                    ).then_inc(cc_sem)
                    nc.gpsimd.wait_ge(cc_sem, 1)

            # ---------------- phase 2: MLP + messages + scatter
            # software-pipelined: window w's messages are emitted after
            # window (w+RA)'s MLP so the per-engine queues hold ~RA windows
            # of AG-independent work while the AllGather runs.
            RA = 2
            r_tiles = {}

            def emit_mlp(w):
                s0b = w * B
                blk0 = 0
                rlist = []
                for sbc in ss:
                    npair = sbc * 128
                    pr0 = (s0b + blk0) * 128
                    h2_l = []
                    for l in range(4):
                        nch = HL[l] // 128
                        h1 = h1p.tile([128, nch, 512], BF16, tag="h1")
                        for g0 in range(0, nch, 2):
                            gsz = min(2, nch - g0)
                            aps = mlpps.tile([128, 2, 512], F32, tag="mlp")
                            for jj in range(gsz):
                                j = g0 + jj
                                nc.tensor.matmul(
                                    aps[:, jj, :npair],
                                    lhsT=a_t[RBOFF[l]:RBOFF[l] + NL[l],
                                             a_offs[l] + j * 128:
                                             a_offs[l] + (j + 1) * 128],
                                    rhs=rbt_t[RBOFF[l]:RBOFF[l] + NL[l],
                                              pr0:pr0 + npair],
                                    start=True, stop=True,
                                    tile_position=(RBOFF[l], 0))
                            nc.scalar.activation(
                                h1[:, g0:g0 + gsz, :npair],
                                aps[:, :gsz, :npair], SILU)
                        h2 = h2p.tile([128, nch, 512], BF16, tag="h2")
                        for g0 in range(0, nch, 2):
                            gsz = min(2, nch - g0)
                            bps = mlpps.tile([128, 2, 512], F32, tag="mlp")
                            for jj in range(gsz):
                                j = g0 + jj
                                for i in range(nch):
                                    nc.tensor.matmul(
                                        bps[:, jj, :npair],
                                        lhsT=b_t[:, b_offs[(l, i, j)]:
                                                 b_offs[(l, i, j)] + 128],
                                        rhs=h1[:, i, :npair],
                                        start=(i == 0), stop=(i == nch - 1))
                            nc.scalar.activation(
                                h2[:, g0:g0 + gsz, :npair],
                                bps[:, :gsz, :npair], SILU)
                        h2_l.append(h2)
                    for b in range(sbc):
                        rps = cps.tile([128, 512], F32, tag="cps")
                        for l in range(4):
                            nch = HL[l] // 128
                            for i in range(nch):
                                nc.tensor.matmul(
                                    rps[:, ROFF[l]:ROFF[l] + KL[l]],
                                    lhsT=h2_l[l][:, i, b * 128:(b + 1) * 128],
                                    rhs=c_t[:, c_offs[(l, i)]:
                                            c_offs[(l, i)] + KL[l]],
                                    start=(i == 0), stop=(i == nch - 1))
                        r_t = rp.tile([128, RTOT], BF16, tag="r")
                        nc.vector.tensor_copy(r_t[:], rps[:, :RTOT])
                        rlist.append(r_t)
                    blk0 += sbc
                r_tiles[w] = rlist

            def emit_msg(w):
                s0b = w * B
                sel_w = selp.tile([128, B, WIN], BF16, tag="sel")
                nc.sync.dma_start(
                    sel_w[:],
                    in_sel[s0b * WIN * 128:(s0b + B) * WIN * 128].rearrange(
                        "(b p c) -> p b c", p=128, c=WIN))
                pooled = plps.tile([128, 1024], F32, tag="pooled")
                for bb in range(B):
                    wb = s0b + bb
                    r_t = r_tiles[w][bb]
                    xg = xgp.tile([128, FTOT], BF16, tag="xg")
                    nc.gpsimd.indirect_dma_start(
                        out=xg[:], out_offset=None, in_=x_full[:],
                        in_offset=bass.IndirectOffsetOnAxis(
                            ap=idx_t[:, wb:wb + 1], axis=0))
                    uve = uvp.tile([128, FTOT], BF16, tag="uve")
                    for l in range(4):
                        m, k = ML[l], KL[l]
                        nc.vector.tensor_tensor(
                            out=uve[:, MKOFF[l]:MKOFF[l] + m * k]
                                .rearrange("p (u k) -> p u k", u=m),
                            in0=ush_t[:, wb, USHOFF[l]:USHOFF[l] + m]
                                .unsqueeze(2).to_broadcast([128, m, k]),
                            in1=r_t[:, ROFF[l]:ROFF[l] + k]
                                .unsqueeze(1).to_broadcast([128, m, k]),
                            op=MUL)
                    msg = msp.tile([128, FTOT], BF16, tag="msg")
                    nc.vector.tensor_tensor(out=msg[:], in0=uve[:],
                                            in1=xg[:], op=MUL)
                    first = (bb == 0)
                    last = (bb == B - 1)
                    nc.tensor.matmul(pooled[:, 0:512],
                                     lhsT=sel_w[:, bb, :],
                                     rhs=msg[:, 0:512],
                                     start=first, stop=last)
                    nc.tensor.matmul(pooled[:, 512:512 + 448],
                                     lhsT=sel_w[:, bb, :],
                                     rhs=msg[:, 512:960],
                                     start=first, stop=last)
                del r_tiles[w]
                pb = pbp.tile([128, FTOT], BF16, tag="pb")
                nc.vector.tensor_copy(pb[:], pooled[:, :FTOT])
                nc.sync.dma_start(pooled_dram[w * 128:(w + 1) * 128, :], pb[:])

            for w in range(WPC):
                emit_mlp(w)
                if w >= RA:
                    emit_msg(w - RA)
            for w in range(WPC - RA, WPC):
                emit_msg(w)
            ph2.close()

            # ---------------- phase 3: out = f + pooled @ BigWout
            with ExitStack() as ph3:
                op = ph3.enter_context(tc.tile_pool(name="op", bufs=3))
                ops = ph3.enter_context(
                    tc.tile_pool(name="ops", bufs=4, space="PSUM"))
                for w in range(WPC):
                    f_t = op.tile([128, FTOT], F32, tag="fo")
                    nc.sync.dma_start(f_t[:], in_f[w * 128:(w + 1) * 128, :])
                    pl_t = op.tile([128, FTOT], BF16, tag="pl")
                    nc.sync.dma_start(pl_t[:],
                                      pooled_dram[w * 128:(w + 1) * 128, :])
                    pT = op.tile([128, 9 * 128], BF16, tag="pT")
                    ci = 0
                    for l in range(4):
                        r0 = MKOFF[l]
                        for s in CHUNKS[l]:
                            tp = ops.tile([128, 128], BF16, tag="tp2")
                            nc.tensor.transpose(out=tp[:s, :],
                                                in_=pl_t[:, r0:r0 + s],
                                                identity=ident_bf[:])
                            nc.vector.tensor_copy(
                                pT[:s, ci * 128:(ci + 1) * 128], tp[:s, :])
                            r0 += s
                            ci += 1
                    out_t = op.tile([128, FTOT], F32, tag="out")
                    ci = 0
                    for l in range(4):
                        yo = ops.tile([128, 512], F32, tag="yo")
                        for k, s in enumerate(CHUNKS[l]):
                            nc.tensor.matmul(
                                yo[:, :MKPAD[l]],
                                lhsT=pT[:s, ci * 128:ci * 128 + 128],
                                rhs=bigwout_t[:s, bw_offs[l][k]:
                                              bw_offs[l][k] + MKPAD[l]],
                                start=(k == 0), stop=(k == len(CHUNKS[l]) - 1))
                            ci += 1
                        nc.vector.tensor_add(
                            out_t[:, MKOFF[l]:MKOFF[l] + MKL[l]],
                            f_t[:, MKOFF[l]:MKOFF[l] + MKL[l]],
                            yo[:, :MKL[l]])
                    nc.sync.dma_start(out_d[w * 128:(w + 1) * 128, :],
                                      out_t[:])

    nc.compile()
    return nc


def kernel(**inputs):
    in_maps, meta = _host_prep(inputs)
    nc = _build_program(meta)
    trace = bool(int(os.environ.get("KERNEL_TRACE", "0")))
    res = run_bass_kernel_spmd(nc, in_maps, list(range(NCORES)), trace=trace)
    if trace and res.exec_time_ns is not None:
        print(f"HW exec time: {res.exec_time_ns} ns")
        kernel.last_exec_time_ns = res.exec_time_ns
    full = np.concatenate([res.results[c]["out"] for c in range(NCORES)],
                          axis=0)[:NATOMS]
    outs = []
    for l in range(4):
        outs.append(np.ascontiguousarray(
            full[:, MKOFF[l]:MKOFF[l] + MKL[l]].reshape(
                NATOMS, ML[l], KL[l]).astype(np.float32)))
    return tuple(outs)
